# revision 27
# baseline (speedup 1.0000x reference)
"""Multi-head attention on 8 Trainium2 NeuronCores.

Problem: x[4,2048,1024] -> attention(16 heads, d=64) -> out proj -> [4,2048,1024].

Sharding: core c handles (batch b = c//2, sequence half s = c%2). Each core
computes q for its 1024 query rows and k/v for the full 2048 rows of its
batch (k/v recomputed by both half-cores — cheaper than a collective), so
cores are fully independent and the host just concatenates outputs.

Per-core dataflow (all matmuls fp32r, PSUM fp32):
  x -> x^T (PE transpose)            [c, rows]
  q^T = Wq_pair^T @ x^T              [128(2 heads), 1024]
  k^T = Wk_pair^T @ x^T              [128(2 heads), 2048]
  v   = x^T.T @ Wv (4-head waves)    [j, 4*65] with a ones column per head
  S^T = k_h^T.T-slices @ q_h^T       [j-block, i]   (K=64)
  expS = exp(S^T * 0.125)            ScalarE, PSUM->SBUF, [128,1024] batches
  out^T_aug = v_aug^T @ expS^T       [65, i]  row 64 = softmax denominator Z
  out^T = out^T_aug[0:64] * (1/Z)    (Z broadcast across partitions via PE)
  y = out^T.T @ Wo + bo              [i, 1024]
"""

import sys

if "/opt/trn_rl_repo" not in sys.path:
    sys.path.insert(0, "/opt/trn_rl_repo")

import numpy as np

B = 4
NSEQ = 2048
C = 1024          # query/model dim
H = 16
DH = 64
NI = 1024         # query rows per core
NJ = 2048         # key rows per core
NCC = C // 128    # 8 contraction chunks
NJB = NJ // 128   # 16 j blocks
SCALE = DH ** -0.5

_CACHE = {}


def _build_program():
    import concourse.bass as bass
    import concourse.mybir as mybir
    import concourse.tile as tile
    from concourse import bacc
    from concourse.masks import make_identity

    f32 = mybir.dt.float32
    f32r = mybir.dt.float32r
    bf16 = mybir.dt.bfloat16
    EXP = mybir.ActivationFunctionType.Exp
    MULT = mybir.AluOpType.mult
    ADD = mybir.AluOpType.add

    nc = bacc.Bacc("TRN2", target_bir_lowering=False, debug=False, num_devices=8)

    x_d = nc.dram_tensor("x", [C, NSEQ], bf16, kind="ExternalInput").ap()
    wq_d = nc.dram_tensor("Wq", [C, H * DH], f32, kind="ExternalInput").ap()
    wk_d = nc.dram_tensor("Wk", [C, H * DH], f32, kind="ExternalInput").ap()
    wv_d = nc.dram_tensor("Wv", [C, H * DH], f32, kind="ExternalInput").ap()
    wo_d = nc.dram_tensor("Wo", [H * DH, C], f32, kind="ExternalInput").ap()
    bo_d = nc.dram_tensor("bo", [C], f32, kind="ExternalInput").ap()
    y_d = nc.dram_tensor("y", [NI, C], f32, kind="ExternalOutput").ap()
    sc_d = nc.dram_tensor("outT_sc", [H, DH, NI], f32).ap()  # out^T bounce

    def r(ap):
        return ap.bitcast(f32r)

    with tile.TileContext(nc) as tc:
        with tc.tile_pool(name="sb", bufs=1) as sbp, \
             tc.tile_pool(name="ps", bufs=1, space="PSUM") as psp:

            # --- constants -------------------------------------------------
            ones_f32 = sbp.tile([128, 128], f32, tag="misc3", bufs=1)
            nc.gpsimd.memset(ones_f32[:], 1.0)
            onest = sbp.tile([128, 64], f32r, tag="misc2", bufs=1)
            nc.vector.tensor_copy(out=onest[:], in_=ones_f32[:, 0:64])
            ones_row = onest[64:65, :]               # [1, 64] at partition 64

            bias = sbp.tile([128, C], f32, tag="bias", bufs=1)
            nc.gpsimd.dma_start(out=bias[:],
                                in_=bo_d[None, :].to_broadcast((128, C)))

            # --- phase 0: load x^T (host pre-transposed) ------------------
            xT = []
            for cc in range(NCC):
                xT.append(sbp.tile([128, NSEQ], bf16, tag="xT", bufs=8, name=f"xT{cc}"))
            for lo, hi in ((0, 512), (512, 1024), (1024, 2048)):
                for cc in range(NCC):
                    nc.sync.dma_start(
                        out=xT[cc][:, lo:hi],
                        in_=x_d[cc * 128:(cc + 1) * 128, lo:hi])

            wq34 = wq_d.rearrange("(cc p) e -> p cc e", p=128)
            wk34 = wk_d.rearrange("(cc p) e -> p cc e", p=128)
            wv34 = wv_d.rearrange("(cc p) e -> p cc e", p=128)

            vq_tiles = {}
            pending_norm = []

            def _make_norm(po, outTh, head, iq):
                def emit():
                    # out^T[0:64] * (1 / Z), Z = po[64]
                    rc = sbp.tile([128, 512], f32r, tag="zb", bufs=4,
                                  name=f"rc{head}_{iq}")
                    with nc.allow_low_precision(reason="fp32r Z broadcast"):
                        nc.vector.reciprocal(rc[64:65, :], po[64:65, :])
                    pz = psp.tile([128, 512], f32, tag="pst", bufs=2,
                                  name=f"pz{head}_{iq}")
                    nc.tensor.matmul(
                        pz[0:64, :], r(ones_row), r(rc[64:65, :]),
                        start=True, stop=True)
                    zb = sbp.tile([128, 512], f32, tag="zb", bufs=4,
                                  name=f"zb{head}_{iq}")
                    nc.vector.tensor_copy(out=zb[0:64, :], in_=pz[0:64, :])
                    nc.vector.tensor_tensor(
                        out=outTh[:, iq * 512:(iq + 1) * 512],
                        in0=po[0:64, :], in1=zb[0:64, :], op=MULT)
                    if iq == 1:
                        nc.sync.dma_start(out=sc_d[head], in_=outTh[:])
                return emit

            qkT_tiles = {}

            wvq_tiles = {}

            def proj_gen(p):
                """Emit pair p's projections in small chunks (generator) so
                they can be interleaved into the previous pair's
                ACT-paced attention loop, keeping the PE array dense.
                The v-wave for qwave qw is split between its two pairs'
                generators (8 j-blocks each) to balance pump work."""
                qw = p // 2
                wqp = sbp.tile([128, C], bf16, tag="wqk", bufs=4,
                               name=f"wqp{p}")
                nc.gpsimd.dma_start(
                    out=wqp[:].rearrange("p (cc e) -> p cc e", cc=8),
                    in_=wq34[:, :, p * 128:(p + 1) * 128])
                wkp = sbp.tile([128, C], bf16, tag="wqk", bufs=4,
                               name=f"wkp{p}")
                nc.gpsimd.dma_start(
                    out=wkp[:].rearrange("p (cc e) -> p cc e", cc=8),
                    in_=wk34[:, :, p * 128:(p + 1) * 128])
                # v waves hold 8 heads (N=512 projections). Wave 0 is
                # fully projected by pair 0 (upfront); wave 1 is created by
                # pair 2 (first 8 j-blocks) and finished by pair 3 -- both
                # drained before attention of pair 4 needs it.
                wv_new = 0 if p == 0 else (1 if p == 2 else None)
                if wv_new is not None:
                    wvq = sbp.tile([128, 8 * 512], bf16, tag="wvq", bufs=2,
                                   name=f"wvq{wv_new}")
                    wvq_tiles[wv_new] = wvq
                    nc.gpsimd.dma_start(
                        out=wvq[:].rearrange("p (cc e) -> p cc e", cc=8),
                        in_=wv34[:, :, wv_new * 512:(wv_new + 1) * 512])
                    vq = sbp.tile([128, NJB * 520], bf16, tag="vq", bufs=2,
                                  name=f"vq{wv_new}")
                    vq_tiles[wv_new] = vq
                    # ones columns (col 64 of each head group)
                    nc.vector.tensor_copy(
                        out=vq[:].rearrange("p (jb h e) -> p jb h e",
                                            jb=NJB, h=8)[:, :, :, 64:65],
                        in_=ones_f32[:].rearrange(
                            "p (a b c) -> p a b c", a=NJB, b=8))
                qT = sbp.tile([128, NI], bf16, tag="qT", bufs=2,
                              name=f"qT{p}")
                kT = sbp.tile([128, NJ], bf16, tag="kT", bufs=2,
                              name=f"kT{p}")
                qkT_tiles[p] = (qT, kT)
                for it in range(NI // 512):
                    pq = psp.tile([128, 512], f32, tag="pst", bufs=2,
                                  name=f"pq{p}_{it}")
                    for cc in range(NCC):
                        nc.tensor.matmul(
                            pq[:], wqp[:, cc * 128:(cc + 1) * 128],
                            xT[cc][:, it * 512:(it + 1) * 512],
                            start=(cc == 0), stop=(cc == NCC - 1))
                        if cc == 3:
                            yield
                    nc.vector.tensor_copy(
                        out=qT[:, it * 512:(it + 1) * 512], in_=pq[:])
                    yield
                for jt in range(NJ // 512):
                    pk = psp.tile([128, 512], f32, tag="pst", bufs=2,
                                  name=f"pk{p}_{jt}")
                    for cc in range(NCC):
                        nc.tensor.matmul(
                            pk[:], wkp[:, cc * 128:(cc + 1) * 128],
                            xT[cc][:, jt * 512:(jt + 1) * 512],
                            start=(cc == 0), stop=(cc == NCC - 1))
                        if cc == 3:
                            yield
                    nc.vector.tensor_copy(
                        out=kT[:, jt * 512:(jt + 1) * 512], in_=pk[:])
                    yield
                if p == 0:
                    vjbs, wv_p = range(NJB), 0
                elif p == 2:
                    vjbs, wv_p = range(0, NJB // 2), 1
                elif p == 3:
                    vjbs, wv_p = range(NJB // 2, NJB), 1
                else:
                    vjbs, wv_p = range(0), None
                if wv_p is not None:
                    vq_w, wvq_w = vq_tiles.get(wv_p), wvq_tiles.get(wv_p)
                for jb in vjbs:
                    pv = psp.tile([128, 512], f32, tag="pst", bufs=2,
                                  name=f"pv{p}_{jb}")
                    for cc in range(NCC):
                        nc.tensor.matmul(
                            pv[:], xT[cc][:, jb * 128:(jb + 1) * 128],
                            wvq_w[:, cc * 512:(cc + 1) * 512],
                            start=(cc == 0), stop=(cc == NCC - 1))
                        if cc == 3:
                            yield
                    nc.vector.tensor_copy(
                        out=vq_w[:].rearrange(
                            "p (jb h e) -> p jb h e", jb=NJB, h=8)
                        [:, jb, :, 0:64],
                        in_=pv[:].rearrange("p (h e) -> p h e", h=8))
                    yield

            gens = {}

            def pump(p):
                g = gens.get(p)
                if g is not None and next(g, "done") == "done":
                    del gens[p]

            def drain(p):
                while p in gens:
                    pump(p)

            wo34 = wo_d.rearrange("(cc p) e -> p cc e", p=128)
            wo_holder = {}
            y_acc = {}

            def phasec_gen():
                """Pass 1 of the output projection (chunks 0..5) into an
                SBUF accumulator, interleaved into pair 7's attention."""
                wo_lo = sbp.tile([128, 4096], bf16, tag="wo", bufs=2)
                wo_hi = sbp.tile([128, 4096], bf16, tag="wo", bufs=2)
                wo_holder["lo"], wo_holder["hi"] = wo_lo, wo_hi
                nc.gpsimd.dma_start(
                    out=wo_lo[:].rearrange("p (cc e) -> p cc e", cc=4),
                    in_=wo34[:, 0:4, :])
                nc.gpsimd.dma_start(
                    out=wo_hi[:].rearrange("p (cc e) -> p cc e", cc=4),
                    in_=wo34[:, 4:8, :])
                outc = []
                for cc in range(6):
                    oc = sbp.tile([128, NI], bf16, tag="xT", bufs=8,
                                  name=f"outc{cc}")
                    nc.gpsimd.dma_start(out=oc[0:64, :], in_=sc_d[2 * cc])
                    nc.gpsimd.dma_start(out=oc[64:128, :],
                                        in_=sc_d[2 * cc + 1])
                    outc.append(oc)
                yield
                for ib2 in range(NI // 128):
                    for eh in range(C // 512):
                        pc = psp.tile([128, 512], f32, tag="pst", bufs=2,
                                      name=f"pc{ib2}_{eh}")
                        for cc in range(6):
                            wo_t = wo_lo if cc < 4 else wo_hi
                            co = cc % 4
                            nc.tensor.matmul(
                                pc[:],
                                outc[cc][:, ib2 * 128:(ib2 + 1) * 128],
                                wo_t[:, co * 1024 + eh * 512:
                                     co * 1024 + eh * 512 + 512],
                                start=(cc == 0), stop=(cc == 5))
                            if cc == 2:
                                yield
                        ya = sbp.tile([128, 512], f32, tag="yacc", bufs=16,
                                      name=f"ya{ib2}_{eh}")
                        nc.vector.tensor_tensor(
                            out=ya[:], in0=pc[:],
                            in1=bias[:, eh * 512:(eh + 1) * 512], op=ADD)
                        y_acc[(ib2, eh)] = ya
                        yield

            gens[0] = proj_gen(0)
            drain(0)

            for p in range(H // 2):          # head pair index
                qw = p // 4
                if p + 1 < H // 2:
                    gens[p + 1] = proj_gen(p + 1)
                else:
                    gens["C"] = phasec_gen()
                qT, kT = qkT_tiles[p]

                # --- attention for the pair's two heads -------------------
                vq = vq_tiles[qw]
                for h2 in range(2):
                    head = 2 * p + h2
                    d0 = h2 * 64
                    hq = (p % 4) * 2 + h2    # head index within the v wave
                    outTh = sbp.tile([64, NI], f32, tag="outTh", bufs=2)
                    for iq in range(NI // 512):
                        po = psp.tile([128, 512], f32, tag="pso", bufs=2)
                        for jbp in range(NJB // 2):
                            ps = psp.tile([128, 1024], f32, tag="pss", bufs=2)
                            for u in range(2):
                                jb = 2 * jbp + u
                                nc.tensor.matmul(
                                    ps[:, u * 512:(u + 1) * 512],
                                    kT[d0:d0 + 64, jb * 128:(jb + 1) * 128],
                                    qT[d0:d0 + 64, iq * 512:(iq + 1) * 512],
                                    start=True, stop=True)
                            ex = sbp.tile([128, 1024], bf16, tag="exps", bufs=3)
                            nc.scalar.activation(ex[:], ps[:], EXP, scale=SCALE)
                            for u in range(2):
                                jb = 2 * jbp + u
                                nc.tensor.matmul(
                                    po[0:65, :],
                                    vq[:, jb * 520 + hq * 65:
                                       jb * 520 + hq * 65 + 65],
                                    ex[:, u * 512:(u + 1) * 512],
                                    start=(jb == 0), stop=(jb == NJB - 1))
                            if jbp in (2, 5) and pending_norm:
                                pending_norm.pop(0)()
                            elif (p + 1) in gens:
                                pump(p + 1)
                            else:
                                pump("C")
                        # defer the normalization so the reciprocal overlaps
                        # the next head's S/av matmuls instead of stalling PE
                        pending_norm.append(_make_norm(po, outTh, head, iq))
                drain(p + 1)

            for fn in pending_norm:
                fn()
            pending_norm = []
            drain("C")

            # --- phase C pass 2: chunks 6..7 + bias ----------------------
            outc67 = []
            for cc in (6, 7):
                oc = sbp.tile([128, NI], bf16, tag="xT", bufs=8,
                              name=f"outc{cc}")
                nc.gpsimd.dma_start(out=oc[0:64, :], in_=sc_d[2 * cc])
                nc.gpsimd.dma_start(out=oc[64:128, :], in_=sc_d[2 * cc + 1])
                outc67.append(oc)
            for ib2 in range(NI // 128):
                for eh in range(C // 512):
                    py = psp.tile([128, 512], f32, tag="pst", bufs=2,
                                  name=f"py{ib2}_{eh}")
                    for ci, cc in enumerate((6, 7)):
                        nc.tensor.matmul(
                            py[:],
                            outc67[ci][:, ib2 * 128:(ib2 + 1) * 128],
                            wo_holder["hi"][:, (cc - 4) * 1024 + eh * 512:
                                            (cc - 4) * 1024 + eh * 512 + 512],
                            start=(ci == 0), stop=(ci == 1))
                    ys = sbp.tile([128, 512], f32, tag="zb", bufs=4,
                                  name=f"ys{ib2}_{eh}")
                    nc.vector.tensor_tensor(
                        out=ys[:], in0=py[:],
                        in1=y_acc[(ib2, eh)][:], op=ADD)
                    nc.sync.dma_start(
                        out=y_d[ib2 * 128:(ib2 + 1) * 128,
                                eh * 512:(eh + 1) * 512],
                        in_=ys[:])

    nc.compile()
    return nc


def _get_program():
    if "nc" not in _CACHE:
        _CACHE["nc"] = _build_program()
    return _CACHE["nc"]


def _make_in_maps(x, Wq, Wk, Wv, Wo, bo):
    import ml_dtypes
    x = np.ascontiguousarray(np.asarray(x, dtype=np.float32))
    Wq = np.ascontiguousarray(np.asarray(Wq, dtype=np.float32))
    Wk = np.ascontiguousarray(np.asarray(Wk, dtype=np.float32))
    Wv = np.ascontiguousarray(np.asarray(Wv, dtype=np.float32))
    Wo = np.ascontiguousarray(np.asarray(Wo, dtype=np.float32))
    bo = np.ascontiguousarray(np.asarray(bo, dtype=np.float32))
    in_maps = []
    for c in range(8):
        b, s = c // 2, c % 2
        # rows 0..1023 of the per-core x are that core's query rows
        xb = x[b]
        xc = np.concatenate(
            [xb[s * NI:(s + 1) * NI], xb[(1 - s) * NI:(2 - s) * NI]], axis=0)
        in_maps.append({"x": np.ascontiguousarray(xc.T.astype(ml_dtypes.bfloat16)),
                        "Wq": Wq, "Wk": Wk, "Wv": Wv, "Wo": Wo, "bo": bo})
    return in_maps


def _assemble(results):
    out = np.empty((B, NSEQ, C), dtype=np.float32)
    for c in range(8):
        b, s = c // 2, c % 2
        out[b, s * NI:(s + 1) * NI] = results[c]["y"]
    return out


def kernel(x, Wq, Wk, Wv, Wo, bo):
    from concourse.bass_utils import run_bass_kernel_spmd

    nc = _get_program()
    in_maps = _make_in_maps(x, Wq, Wk, Wv, Wo, bo)
    res = run_bass_kernel_spmd(nc, in_maps, list(range(8)))
    return _assemble(res.results)


def kernel_traced(x, Wq, Wk, Wv, Wo, bo):
    """Like kernel() but also neuron-profiles; returns (out, exec_time_ns)."""
    from concourse.bass_utils import run_bass_kernel_spmd

    nc = _get_program()
    in_maps = _make_in_maps(x, Wq, Wk, Wv, Wo, bo)
    res = run_bass_kernel_spmd(nc, in_maps, list(range(8)), trace=True)
    return _assemble(res.results), res.exec_time_ns


# revision 28
# speedup vs baseline: 1.0759x; 1.0759x over previous
"""Multi-head attention on 8 Trainium2 NeuronCores.

Problem: x[4,2048,1024] -> attention(16 heads, d=64) -> out proj -> [4,2048,1024].

Sharding: core c handles (batch b = c//2, sequence half s = c%2). Each core
computes q for its 1024 query rows and k/v for the full 2048 rows of its
batch (k/v recomputed by both half-cores — cheaper than a collective), so
cores are fully independent and the host just concatenates outputs.

Per-core dataflow (all matmuls fp32r, PSUM fp32):
  x -> x^T (PE transpose)            [c, rows]
  q^T = Wq_pair^T @ x^T              [128(2 heads), 1024]
  k^T = Wk_pair^T @ x^T              [128(2 heads), 2048]
  v   = x^T.T @ Wv (4-head waves)    [j, 4*65] with a ones column per head
  S^T = k_h^T.T-slices @ q_h^T       [j-block, i]   (K=64)
  expS = exp(S^T * 0.125)            ScalarE, PSUM->SBUF, [128,1024] batches
  out^T_aug = v_aug^T @ expS^T       [65, i]  row 64 = softmax denominator Z
  out^T = out^T_aug[0:64] * (1/Z)    (Z broadcast across partitions via PE)
  y = out^T.T @ Wo + bo              [i, 1024]
"""

import sys

if "/opt/trn_rl_repo" not in sys.path:
    sys.path.insert(0, "/opt/trn_rl_repo")

import numpy as np

B = 4
NSEQ = 2048
C = 1024          # query/model dim
H = 16
DH = 64
NI = 1024         # query rows per core
NJ = 2048         # key rows per core
NCC = C // 128    # 8 contraction chunks
NJB = NJ // 128   # 16 j blocks
SCALE = DH ** -0.5

_CACHE = {}


def _build_program():
    import concourse.bass as bass
    import concourse.mybir as mybir
    import concourse.tile as tile
    from concourse import bacc
    from concourse.masks import make_identity

    f32 = mybir.dt.float32
    f32r = mybir.dt.float32r
    bf16 = mybir.dt.bfloat16
    EXP = mybir.ActivationFunctionType.Exp
    MULT = mybir.AluOpType.mult
    ADD = mybir.AluOpType.add

    nc = bacc.Bacc("TRN2", target_bir_lowering=False, debug=False, num_devices=8)

    x_d = nc.dram_tensor("x", [C, NSEQ], bf16, kind="ExternalInput").ap()
    wq_d = nc.dram_tensor("Wq", [C, H * DH], f32, kind="ExternalInput").ap()
    wk_d = nc.dram_tensor("Wk", [C, H * DH], f32, kind="ExternalInput").ap()
    wv_d = nc.dram_tensor("Wv", [C, H * DH], f32, kind="ExternalInput").ap()
    wo_d = nc.dram_tensor("Wo", [H * DH, C], f32, kind="ExternalInput").ap()
    bo_d = nc.dram_tensor("bo", [C], f32, kind="ExternalInput").ap()
    y_d = nc.dram_tensor("y", [NI, C], f32, kind="ExternalOutput").ap()
    sc_d = nc.dram_tensor("outT_sc", [H, DH, NI], f32).ap()  # out^T bounce

    def r(ap):
        return ap.bitcast(f32r)

    with tile.TileContext(nc) as tc:
        with tc.tile_pool(name="sb", bufs=1) as sbp, \
             tc.tile_pool(name="ps", bufs=1, space="PSUM") as psp:

            # --- constants -------------------------------------------------
            ones_f32 = sbp.tile([128, 128], f32, tag="misc3", bufs=1)
            nc.gpsimd.memset(ones_f32[:], 1.0)
            onest = sbp.tile([128, 64], f32r, tag="misc2", bufs=1)
            nc.vector.tensor_copy(out=onest[:], in_=ones_f32[:, 0:64])
            ones_row = onest[64:65, :]               # [1, 64] at partition 64

            bias = sbp.tile([128, C], f32, tag="bias", bufs=1)
            nc.gpsimd.dma_start(out=bias[:],
                                in_=bo_d[None, :].to_broadcast((128, C)))

            # --- phase 0: load x^T (host pre-transposed) ------------------
            xT = []
            for cc in range(NCC):
                xT.append(sbp.tile([128, NSEQ], bf16, tag="xT", bufs=8, name=f"xT{cc}"))
            for lo, hi in ((0, 512), (512, 1024), (1024, 2048)):
                for cc in range(NCC):
                    nc.sync.dma_start(
                        out=xT[cc][:, lo:hi],
                        in_=x_d[cc * 128:(cc + 1) * 128, lo:hi])

            wq34 = wq_d.rearrange("(cc p) e -> p cc e", p=128)
            wk34 = wk_d.rearrange("(cc p) e -> p cc e", p=128)
            wv34 = wv_d.rearrange("(cc p) e -> p cc e", p=128)

            vq_tiles = {}
            pending_norm = []

            def _make_norm(po, outTh, head, iq):
                def emit():
                    # out^T[0:64] * (1 / Z), Z = po[64]
                    rc = sbp.tile([128, 512], f32r, tag="zb", bufs=4,
                                  name=f"rc{head}_{iq}")
                    with nc.allow_low_precision(reason="fp32r Z broadcast"):
                        nc.vector.reciprocal(rc[64:65, :], po[64:65, :])
                    pz = psp.tile([128, 512], f32, tag="pst", bufs=2,
                                  name=f"pz{head}_{iq}")
                    nc.tensor.matmul(
                        pz[0:64, :], r(ones_row), r(rc[64:65, :]),
                        start=True, stop=True)
                    zb = sbp.tile([128, 512], f32, tag="zb", bufs=4,
                                  name=f"zb{head}_{iq}")
                    nc.vector.tensor_copy(out=zb[0:64, :], in_=pz[0:64, :])
                    nc.vector.tensor_tensor(
                        out=outTh[:, iq * 512:(iq + 1) * 512],
                        in0=po[0:64, :], in1=zb[0:64, :], op=MULT)
                    if iq == 1:
                        nc.sync.dma_start(out=sc_d[head], in_=outTh[:])
                return emit

            qkT_tiles = {}

            wvq_tiles = {}

            def proj_gen(p):
                """Emit pair p's projections in small chunks (generator) so
                they can be interleaved into the previous pair's
                ACT-paced attention loop, keeping the PE array dense.
                The v-wave for qwave qw is split between its two pairs'
                generators (8 j-blocks each) to balance pump work."""
                qw = p // 2
                wqp = sbp.tile([128, C], bf16, tag="wqk", bufs=4,
                               name=f"wqp{p}")
                nc.gpsimd.dma_start(
                    out=wqp[:].rearrange("p (cc e) -> p cc e", cc=8),
                    in_=wq34[:, :, p * 128:(p + 1) * 128])
                wkp = sbp.tile([128, C], bf16, tag="wqk", bufs=4,
                               name=f"wkp{p}")
                nc.gpsimd.dma_start(
                    out=wkp[:].rearrange("p (cc e) -> p cc e", cc=8),
                    in_=wk34[:, :, p * 128:(p + 1) * 128])
                # wave ownership: pair 0 creates+fully projects wave 0;
                # afterwards odd pair p creates wave (p+1)//2 and projects
                # its first half, even pair p>0 completes wave p//2.
                wv_new = 0 if p == 0 else ((p + 1) // 2 if p % 2 == 1 else None)
                if wv_new is not None and wv_new < H // 4:
                    wvq = sbp.tile([128, 8 * 256], bf16, tag="wvq", bufs=2,
                                   name=f"wvq{wv_new}")
                    wvq_tiles[wv_new] = wvq
                    nc.gpsimd.dma_start(
                        out=wvq[:].rearrange("p (cc e) -> p cc e", cc=8),
                        in_=wv34[:, :, wv_new * 256:(wv_new + 1) * 256])
                    vq = sbp.tile([128, NJB * 260], bf16, tag="vq", bufs=2,
                                  name=f"vq{wv_new}")
                    vq_tiles[wv_new] = vq
                    # ones columns (col 64 of each head group)
                    nc.vector.tensor_copy(
                        out=vq[:].rearrange("p (jb h e) -> p jb h e",
                                            jb=NJB, h=4)[:, :, :, 64:65],
                        in_=ones_f32[:, 0:64].rearrange(
                            "p (a b c) -> p a b c", a=NJB, b=4))
                qT = sbp.tile([128, NI], bf16, tag="qT", bufs=2,
                              name=f"qT{p}")
                kT = sbp.tile([128, NJ], bf16, tag="kT", bufs=2,
                              name=f"kT{p}")
                qkT_tiles[p] = (qT, kT)
                for it in range(NI // 512):
                    pq = psp.tile([128, 512], f32, tag="pst", bufs=2,
                                  name=f"pq{p}_{it}")
                    for cc in range(NCC):
                        nc.tensor.matmul(
                            pq[:], wqp[:, cc * 128:(cc + 1) * 128],
                            xT[cc][:, it * 512:(it + 1) * 512],
                            start=(cc == 0), stop=(cc == NCC - 1))
                        if cc == 3:
                            yield
                    nc.vector.tensor_copy(
                        out=qT[:, it * 512:(it + 1) * 512], in_=pq[:])
                    yield
                for jt in range(NJ // 512):
                    pk = psp.tile([128, 512], f32, tag="pst", bufs=2,
                                  name=f"pk{p}_{jt}")
                    for cc in range(NCC):
                        nc.tensor.matmul(
                            pk[:], wkp[:, cc * 128:(cc + 1) * 128],
                            xT[cc][:, jt * 512:(jt + 1) * 512],
                            start=(cc == 0), stop=(cc == NCC - 1))
                        if cc == 3:
                            yield
                    nc.vector.tensor_copy(
                        out=kT[:, jt * 512:(jt + 1) * 512], in_=pk[:])
                    yield
                if p == 0:
                    vjbs, wv_p = range(NJB), 0
                elif p % 2 == 1 and (p + 1) // 2 < H // 4:
                    vjbs, wv_p = range(0, NJB // 2), (p + 1) // 2
                elif p % 2 == 0:
                    vjbs, wv_p = range(NJB // 2, NJB), p // 2
                else:
                    vjbs, wv_p = range(0), None
                if wv_p is not None:
                    vq_w, wvq_w = vq_tiles.get(wv_p), wvq_tiles.get(wv_p)
                for jb in vjbs:
                    pv = psp.tile([128, 256], f32, tag="pst", bufs=2,
                                  name=f"pv{p}_{jb}")
                    for cc in range(NCC):
                        nc.tensor.matmul(
                            pv[:], xT[cc][:, jb * 128:(jb + 1) * 128],
                            wvq_w[:, cc * 256:(cc + 1) * 256],
                            start=(cc == 0), stop=(cc == NCC - 1))
                        if cc == 3:
                            yield
                    nc.vector.tensor_copy(
                        out=vq_w[:].rearrange(
                            "p (jb h e) -> p jb h e", jb=NJB, h=4)
                        [:, jb, :, 0:64],
                        in_=pv[:].rearrange("p (h e) -> p h e", h=4))
                    yield

            gens = {}

            def pump(p):
                g = gens.get(p)
                if g is not None and next(g, "done") == "done":
                    del gens[p]

            def drain(p):
                while p in gens:
                    pump(p)

            wo34 = wo_d.rearrange("(cc p) e -> p cc e", p=128)
            wo_holder = {}
            y_acc = {}

            def phasec_gen():
                """Pass 1 of the output projection (chunks 0..5) into an
                SBUF accumulator, interleaved into pair 7's attention."""
                wo_lo = sbp.tile([128, 4096], bf16, tag="wo", bufs=2)
                wo_hi = sbp.tile([128, 4096], bf16, tag="wo", bufs=2)
                wo_holder["lo"], wo_holder["hi"] = wo_lo, wo_hi
                nc.gpsimd.dma_start(
                    out=wo_lo[:].rearrange("p (cc e) -> p cc e", cc=4),
                    in_=wo34[:, 0:4, :])
                nc.gpsimd.dma_start(
                    out=wo_hi[:].rearrange("p (cc e) -> p cc e", cc=4),
                    in_=wo34[:, 4:8, :])
                outc = []
                for cc in range(6):
                    oc = sbp.tile([128, NI], bf16, tag="xT", bufs=8,
                                  name=f"outc{cc}")
                    nc.gpsimd.dma_start(out=oc[0:64, :], in_=sc_d[2 * cc])
                    nc.gpsimd.dma_start(out=oc[64:128, :],
                                        in_=sc_d[2 * cc + 1])
                    outc.append(oc)
                yield
                for ib2 in range(NI // 128):
                    for eh in range(C // 512):
                        pc = psp.tile([128, 512], f32, tag="pst", bufs=2,
                                      name=f"pc{ib2}_{eh}")
                        for cc in range(6):
                            wo_t = wo_lo if cc < 4 else wo_hi
                            co = cc % 4
                            nc.tensor.matmul(
                                pc[:],
                                outc[cc][:, ib2 * 128:(ib2 + 1) * 128],
                                wo_t[:, co * 1024 + eh * 512:
                                     co * 1024 + eh * 512 + 512],
                                start=(cc == 0), stop=(cc == 5))
                            if cc == 2:
                                yield
                        ya = sbp.tile([128, 512], f32, tag="yacc", bufs=16,
                                      name=f"ya{ib2}_{eh}")
                        nc.vector.tensor_tensor(
                            out=ya[:], in0=pc[:],
                            in1=bias[:, eh * 512:(eh + 1) * 512], op=ADD)
                        y_acc[(ib2, eh)] = ya
                        yield

            gens[0] = proj_gen(0)
            drain(0)

            for p in range(H // 2):          # head pair index
                qw = p // 2
                if p + 1 < H // 2:
                    gens[p + 1] = proj_gen(p + 1)
                else:
                    gens["C"] = phasec_gen()
                qT, kT = qkT_tiles[p]

                # --- attention for the pair's two heads -------------------
                vq = vq_tiles[qw]
                for h2 in range(2):
                    head = 2 * p + h2
                    d0 = h2 * 64
                    hq = (p % 2) * 2 + h2    # head index within the v wave
                    outTh = sbp.tile([64, NI], f32, tag="outTh", bufs=2)
                    for iq in range(NI // 512):
                        po = psp.tile([128, 512], f32, tag="pso", bufs=2)
                        for jbp in range(NJB // 2):
                            ps = psp.tile([128, 1024], f32, tag="pss", bufs=2)
                            for u in range(2):
                                jb = 2 * jbp + u
                                nc.tensor.matmul(
                                    ps[:, u * 512:(u + 1) * 512],
                                    kT[d0:d0 + 64, jb * 128:(jb + 1) * 128],
                                    qT[d0:d0 + 64, iq * 512:(iq + 1) * 512],
                                    start=True, stop=True)
                            ex = sbp.tile([128, 1024], bf16, tag="exps", bufs=3)
                            nc.scalar.activation(ex[:], ps[:], EXP, scale=SCALE)
                            for u in range(2):
                                jb = 2 * jbp + u
                                nc.tensor.matmul(
                                    po[0:65, :],
                                    vq[:, jb * 260 + hq * 65:
                                       jb * 260 + hq * 65 + 65],
                                    ex[:, u * 512:(u + 1) * 512],
                                    start=(jb == 0), stop=(jb == NJB - 1))
                            if jbp in (2, 5) and pending_norm:
                                pending_norm.pop(0)()
                            elif (p + 1) in gens:
                                pump(p + 1)
                            else:
                                pump("C")
                        # defer the normalization so the reciprocal overlaps
                        # the next head's S/av matmuls instead of stalling PE
                        pending_norm.append(_make_norm(po, outTh, head, iq))
                drain(p + 1)

            for fn in pending_norm:
                fn()
            pending_norm = []
            drain("C")

            # --- phase C pass 2: chunks 6..7 + bias ----------------------
            outc67 = []
            for cc in (6, 7):
                oc = sbp.tile([128, NI], bf16, tag="xT", bufs=8,
                              name=f"outc{cc}")
                nc.gpsimd.dma_start(out=oc[0:64, :], in_=sc_d[2 * cc])
                nc.gpsimd.dma_start(out=oc[64:128, :], in_=sc_d[2 * cc + 1])
                outc67.append(oc)
            for ib2 in range(NI // 128):
                for eh in range(C // 512):
                    py = psp.tile([128, 512], f32, tag="pst", bufs=2,
                                  name=f"py{ib2}_{eh}")
                    for ci, cc in enumerate((6, 7)):
                        nc.tensor.matmul(
                            py[:],
                            outc67[ci][:, ib2 * 128:(ib2 + 1) * 128],
                            wo_holder["hi"][:, (cc - 4) * 1024 + eh * 512:
                                            (cc - 4) * 1024 + eh * 512 + 512],
                            start=(ci == 0), stop=(ci == 1))
                    ys = sbp.tile([128, 512], f32, tag="zb", bufs=4,
                                  name=f"ys{ib2}_{eh}")
                    nc.vector.tensor_tensor(
                        out=ys[:], in0=py[:],
                        in1=y_acc[(ib2, eh)][:], op=ADD)
                    nc.sync.dma_start(
                        out=y_d[ib2 * 128:(ib2 + 1) * 128,
                                eh * 512:(eh + 1) * 512],
                        in_=ys[:])

    nc.compile()
    return nc


def _get_program():
    if "nc" not in _CACHE:
        _CACHE["nc"] = _build_program()
    return _CACHE["nc"]


def _make_in_maps(x, Wq, Wk, Wv, Wo, bo):
    import ml_dtypes
    x = np.ascontiguousarray(np.asarray(x, dtype=np.float32))
    Wq = np.ascontiguousarray(np.asarray(Wq, dtype=np.float32))
    Wk = np.ascontiguousarray(np.asarray(Wk, dtype=np.float32))
    Wv = np.ascontiguousarray(np.asarray(Wv, dtype=np.float32))
    Wo = np.ascontiguousarray(np.asarray(Wo, dtype=np.float32))
    bo = np.ascontiguousarray(np.asarray(bo, dtype=np.float32))
    in_maps = []
    for c in range(8):
        b, s = c // 2, c % 2
        # rows 0..1023 of the per-core x are that core's query rows
        xb = x[b]
        xc = np.concatenate(
            [xb[s * NI:(s + 1) * NI], xb[(1 - s) * NI:(2 - s) * NI]], axis=0)
        in_maps.append({"x": np.ascontiguousarray(xc.T.astype(ml_dtypes.bfloat16)),
                        "Wq": Wq, "Wk": Wk, "Wv": Wv, "Wo": Wo, "bo": bo})
    return in_maps


def _assemble(results):
    out = np.empty((B, NSEQ, C), dtype=np.float32)
    for c in range(8):
        b, s = c // 2, c % 2
        out[b, s * NI:(s + 1) * NI] = results[c]["y"]
    return out


def kernel(x, Wq, Wk, Wv, Wo, bo):
    from concourse.bass_utils import run_bass_kernel_spmd

    nc = _get_program()
    in_maps = _make_in_maps(x, Wq, Wk, Wv, Wo, bo)
    res = run_bass_kernel_spmd(nc, in_maps, list(range(8)))
    return _assemble(res.results)


def kernel_traced(x, Wq, Wk, Wv, Wo, bo):
    """Like kernel() but also neuron-profiles; returns (out, exec_time_ns)."""
    from concourse.bass_utils import run_bass_kernel_spmd

    nc = _get_program()
    in_maps = _make_in_maps(x, Wq, Wk, Wv, Wo, bo)
    res = run_bass_kernel_spmd(nc, in_maps, list(range(8)), trace=True)
    return _assemble(res.results), res.exec_time_ns


# revision 29
# speedup vs baseline: 1.1575x; 1.0758x over previous
"""Multi-head attention on 8 Trainium2 NeuronCores.

Problem: x[4,2048,1024] -> attention(16 heads, d=64) -> out proj -> [4,2048,1024].

Sharding: core c handles (batch b = c//2, sequence half s = c%2). Each core
computes q for its 1024 query rows and k/v for the full 2048 rows of its
batch (k/v recomputed by both half-cores — cheaper than a collective), so
cores are fully independent and the host just concatenates outputs.

Per-core dataflow (all matmuls fp32r, PSUM fp32):
  x -> x^T (PE transpose)            [c, rows]
  q^T = Wq_pair^T @ x^T              [128(2 heads), 1024]
  k^T = Wk_pair^T @ x^T              [128(2 heads), 2048]
  v   = x^T.T @ Wv (4-head waves)    [j, 4*65] with a ones column per head
  S^T = k_h^T.T-slices @ q_h^T       [j-block, i]   (K=64)
  expS = exp(S^T * 0.125)            ScalarE, PSUM->SBUF, [128,1024] batches
  out^T_aug = v_aug^T @ expS^T       [65, i]  row 64 = softmax denominator Z
  out^T = out^T_aug[0:64] * (1/Z)    (Z broadcast across partitions via PE)
  y = out^T.T @ Wo + bo              [i, 1024]
"""

import sys

if "/opt/trn_rl_repo" not in sys.path:
    sys.path.insert(0, "/opt/trn_rl_repo")

import numpy as np

B = 4
NSEQ = 2048
C = 1024          # query/model dim
H = 16
DH = 64
NI = 1024         # query rows per core
NJ = 2048         # key rows per core
NCC = C // 128    # 8 contraction chunks
NJB = NJ // 128   # 16 j blocks
SCALE = DH ** -0.5

_CACHE = {}


def _build_program():
    import concourse.bass as bass
    import concourse.mybir as mybir
    import concourse.tile as tile
    from concourse import bacc
    from concourse.masks import make_identity

    f32 = mybir.dt.float32
    f32r = mybir.dt.float32r
    bf16 = mybir.dt.bfloat16
    EXP = mybir.ActivationFunctionType.Exp
    MULT = mybir.AluOpType.mult
    ADD = mybir.AluOpType.add

    nc = bacc.Bacc("TRN2", target_bir_lowering=False, debug=False, num_devices=8)

    x_d = nc.dram_tensor("x", [C, NSEQ], bf16, kind="ExternalInput").ap()
    wq_d = nc.dram_tensor("Wq", [C, H * DH], f32, kind="ExternalInput").ap()
    wk_d = nc.dram_tensor("Wk", [C, H * DH], f32, kind="ExternalInput").ap()
    wv_d = nc.dram_tensor("Wv", [C, H * DH], f32, kind="ExternalInput").ap()
    wo_d = nc.dram_tensor("Wo", [H * DH, C], f32, kind="ExternalInput").ap()
    bo_d = nc.dram_tensor("bo", [C], f32, kind="ExternalInput").ap()
    y_d = nc.dram_tensor("y", [NI, C], f32, kind="ExternalOutput").ap()
    sc_d = nc.dram_tensor("outT_sc", [H, DH, NI], f32).ap()  # out^T bounce

    def r(ap):
        return ap.bitcast(f32r)

    with tile.TileContext(nc) as tc:
        with tc.tile_pool(name="sb", bufs=1) as sbp, \
             tc.tile_pool(name="ps", bufs=1, space="PSUM") as psp:

            # --- constants -------------------------------------------------
            ones_f32 = sbp.tile([128, 128], f32, tag="misc3", bufs=1)
            nc.gpsimd.memset(ones_f32[:], 1.0)
            onest = sbp.tile([128, 64], f32r, tag="misc2", bufs=1)
            nc.vector.tensor_copy(out=onest[:], in_=ones_f32[:, 0:64])
            ones_row = onest[64:65, :]               # [1, 64] at partition 64

            bias = sbp.tile([128, C], f32, tag="bias", bufs=1)
            nc.gpsimd.dma_start(out=bias[:],
                                in_=bo_d[None, :].to_broadcast((128, C)))

            # --- phase 0: load x^T (host pre-transposed) ------------------
            xT = []
            for cc in range(NCC):
                xT.append(sbp.tile([128, NSEQ], bf16, tag="xT", bufs=8, name=f"xT{cc}"))
            for lo, hi in ((0, 512), (512, 1024), (1024, 2048)):
                for cc in range(NCC):
                    nc.sync.dma_start(
                        out=xT[cc][:, lo:hi],
                        in_=x_d[cc * 128:(cc + 1) * 128, lo:hi])

            wq34 = wq_d.rearrange("(cc p) e -> p cc e", p=128)
            wk34 = wk_d.rearrange("(cc p) e -> p cc e", p=128)
            wv34 = wv_d.rearrange("(cc p) e -> p cc e", p=128)

            vq_tiles = {}
            pending_norm = []

            def _make_norm(po, outTh, head, iq):
                def emit():
                    # out^T[0:64] * (1 / Z), Z = po[64]
                    rc = sbp.tile([128, 512], f32r, tag="zb", bufs=4,
                                  name=f"rc{head}_{iq}")
                    # approx recip over rows 0:65 (custom DVE op needs a
                    # base-0 partition range); only row 64 (= Z) is used
                    rf = sbp.tile([128, 512], f32, tag="zb", bufs=4,
                                  name=f"rf{head}_{iq}")
                    nc.vector.reciprocal_approx_fast(
                        out=rf[0:65, :], in_=po[0:65, :])
                    nc.vector.tensor_copy(out=rc[64:65, :], in_=rf[64:65, :])
                    pz = psp.tile([128, 512], f32, tag="pst", bufs=2,
                                  name=f"pz{head}_{iq}")
                    nc.tensor.matmul(
                        pz[0:64, :], r(ones_row), r(rc[64:65, :]),
                        start=True, stop=True)
                    zb = sbp.tile([128, 512], f32, tag="zb", bufs=4,
                                  name=f"zb{head}_{iq}")
                    nc.vector.tensor_copy(out=zb[0:64, :], in_=pz[0:64, :])
                    nc.vector.tensor_tensor(
                        out=outTh[:, iq * 512:(iq + 1) * 512],
                        in0=po[0:64, :], in1=zb[0:64, :], op=MULT)
                    if iq == 1:
                        nc.sync.dma_start(out=sc_d[head], in_=outTh[:])
                return emit

            qkT_tiles = {}

            wvq_tiles = {}

            def proj_gen(p):
                """Emit pair p's projections in small chunks (generator) so
                they can be interleaved into the previous pair's
                ACT-paced attention loop, keeping the PE array dense.
                The v-wave for qwave qw is split between its two pairs'
                generators (8 j-blocks each) to balance pump work."""
                qw = p // 2
                wqp = sbp.tile([128, C], bf16, tag="wqk", bufs=4,
                               name=f"wqp{p}")
                nc.gpsimd.dma_start(
                    out=wqp[:].rearrange("p (cc e) -> p cc e", cc=8),
                    in_=wq34[:, :, p * 128:(p + 1) * 128])
                wkp = sbp.tile([128, C], bf16, tag="wqk", bufs=4,
                               name=f"wkp{p}")
                nc.gpsimd.dma_start(
                    out=wkp[:].rearrange("p (cc e) -> p cc e", cc=8),
                    in_=wk34[:, :, p * 128:(p + 1) * 128])
                # wave ownership: pair 0 creates+fully projects wave 0;
                # afterwards odd pair p creates wave (p+1)//2 and projects
                # its first half, even pair p>0 completes wave p//2.
                wv_new = 0 if p == 0 else ((p + 1) // 2 if p % 2 == 1 else None)
                if wv_new is not None and wv_new < H // 4:
                    wvq = sbp.tile([128, 8 * 256], bf16, tag="wvq", bufs=2,
                                   name=f"wvq{wv_new}")
                    wvq_tiles[wv_new] = wvq
                    nc.gpsimd.dma_start(
                        out=wvq[:].rearrange("p (cc e) -> p cc e", cc=8),
                        in_=wv34[:, :, wv_new * 256:(wv_new + 1) * 256])
                    vq = sbp.tile([128, NJB * 260], bf16, tag="vq", bufs=2,
                                  name=f"vq{wv_new}")
                    vq_tiles[wv_new] = vq
                    # ones columns (col 64 of each head group)
                    nc.vector.tensor_copy(
                        out=vq[:].rearrange("p (jb h e) -> p jb h e",
                                            jb=NJB, h=4)[:, :, :, 64:65],
                        in_=ones_f32[:, 0:64].rearrange(
                            "p (a b c) -> p a b c", a=NJB, b=4))
                qT = sbp.tile([128, NI], bf16, tag="qT", bufs=2,
                              name=f"qT{p}")
                kT = sbp.tile([128, NJ], bf16, tag="kT", bufs=2,
                              name=f"kT{p}")
                qkT_tiles[p] = (qT, kT)
                for it in range(NI // 512):
                    pq = psp.tile([128, 512], f32, tag="pst", bufs=2,
                                  name=f"pq{p}_{it}")
                    for cc in range(NCC):
                        nc.tensor.matmul(
                            pq[:], wqp[:, cc * 128:(cc + 1) * 128],
                            xT[cc][:, it * 512:(it + 1) * 512],
                            start=(cc == 0), stop=(cc == NCC - 1))
                        if cc == 3:
                            yield
                    nc.vector.tensor_copy(
                        out=qT[:, it * 512:(it + 1) * 512], in_=pq[:])
                    yield
                for jt in range(NJ // 512):
                    pk = psp.tile([128, 512], f32, tag="pst", bufs=2,
                                  name=f"pk{p}_{jt}")
                    for cc in range(NCC):
                        nc.tensor.matmul(
                            pk[:], wkp[:, cc * 128:(cc + 1) * 128],
                            xT[cc][:, jt * 512:(jt + 1) * 512],
                            start=(cc == 0), stop=(cc == NCC - 1))
                        if cc == 3:
                            yield
                    nc.vector.tensor_copy(
                        out=kT[:, jt * 512:(jt + 1) * 512], in_=pk[:])
                    yield
                if p == 0:
                    vjbs, wv_p = range(NJB), 0
                elif p % 2 == 1 and (p + 1) // 2 < H // 4:
                    vjbs, wv_p = range(0, NJB // 2), (p + 1) // 2
                elif p % 2 == 0:
                    vjbs, wv_p = range(NJB // 2, NJB), p // 2
                else:
                    vjbs, wv_p = range(0), None
                if wv_p is not None:
                    vq_w, wvq_w = vq_tiles.get(wv_p), wvq_tiles.get(wv_p)
                for jb in vjbs:
                    pv = psp.tile([128, 256], f32, tag="pst", bufs=2,
                                  name=f"pv{p}_{jb}")
                    for cc in range(NCC):
                        nc.tensor.matmul(
                            pv[:], xT[cc][:, jb * 128:(jb + 1) * 128],
                            wvq_w[:, cc * 256:(cc + 1) * 256],
                            start=(cc == 0), stop=(cc == NCC - 1))
                        if cc == 3:
                            yield
                    nc.vector.tensor_copy(
                        out=vq_w[:].rearrange(
                            "p (jb h e) -> p jb h e", jb=NJB, h=4)
                        [:, jb, :, 0:64],
                        in_=pv[:].rearrange("p (h e) -> p h e", h=4))
                    yield

            gens = {}

            def pump(p):
                g = gens.get(p)
                if g is not None and next(g, "done") == "done":
                    del gens[p]

            def drain(p):
                while p in gens:
                    pump(p)

            wo34 = wo_d.rearrange("(cc p) e -> p cc e", p=128)
            wo_holder = {}
            y_acc = {}

            def phasec_gen():
                """Pass 1 of the output projection (chunks 0..5) into an
                SBUF accumulator, interleaved into pair 7's attention."""
                wo_lo = sbp.tile([128, 4096], bf16, tag="wo", bufs=2)
                wo_hi = sbp.tile([128, 4096], bf16, tag="wo", bufs=2)
                wo_holder["lo"], wo_holder["hi"] = wo_lo, wo_hi
                nc.gpsimd.dma_start(
                    out=wo_lo[:].rearrange("p (cc e) -> p cc e", cc=4),
                    in_=wo34[:, 0:4, :])
                nc.gpsimd.dma_start(
                    out=wo_hi[:].rearrange("p (cc e) -> p cc e", cc=4),
                    in_=wo34[:, 4:8, :])
                outc = []
                for cc in range(6):
                    oc = sbp.tile([128, NI], bf16, tag="xT", bufs=8,
                                  name=f"outc{cc}")
                    nc.gpsimd.dma_start(out=oc[0:64, :], in_=sc_d[2 * cc])
                    nc.gpsimd.dma_start(out=oc[64:128, :],
                                        in_=sc_d[2 * cc + 1])
                    outc.append(oc)
                yield
                for ib2 in range(NI // 128):
                    for eh in range(C // 512):
                        pc = psp.tile([128, 512], f32, tag="pst", bufs=2,
                                      name=f"pc{ib2}_{eh}")
                        for cc in range(6):
                            wo_t = wo_lo if cc < 4 else wo_hi
                            co = cc % 4
                            nc.tensor.matmul(
                                pc[:],
                                outc[cc][:, ib2 * 128:(ib2 + 1) * 128],
                                wo_t[:, co * 1024 + eh * 512:
                                     co * 1024 + eh * 512 + 512],
                                start=(cc == 0), stop=(cc == 5))
                            if cc == 2:
                                yield
                        ya = sbp.tile([128, 512], f32, tag="yacc", bufs=16,
                                      name=f"ya{ib2}_{eh}")
                        nc.vector.tensor_tensor(
                            out=ya[:], in0=pc[:],
                            in1=bias[:, eh * 512:(eh + 1) * 512], op=ADD)
                        y_acc[(ib2, eh)] = ya
                        yield

            gens[0] = proj_gen(0)
            drain(0)

            for p in range(H // 2):          # head pair index
                qw = p // 2
                if p + 1 < H // 2:
                    gens[p + 1] = proj_gen(p + 1)
                else:
                    gens["C"] = phasec_gen()
                qT, kT = qkT_tiles[p]

                # --- attention for the pair's two heads -------------------
                vq = vq_tiles[qw]
                for h2 in range(2):
                    head = 2 * p + h2
                    d0 = h2 * 64
                    hq = (p % 2) * 2 + h2    # head index within the v wave
                    outTh = sbp.tile([64, NI], f32, tag="outTh", bufs=2)
                    for iq in range(NI // 512):
                        po = psp.tile([128, 512], f32, tag="pso", bufs=2)
                        for jbp in range(NJB // 2):
                            ps = psp.tile([128, 1024], f32, tag="pss", bufs=2)
                            for u in range(2):
                                jb = 2 * jbp + u
                                nc.tensor.matmul(
                                    ps[:, u * 512:(u + 1) * 512],
                                    kT[d0:d0 + 64, jb * 128:(jb + 1) * 128],
                                    qT[d0:d0 + 64, iq * 512:(iq + 1) * 512],
                                    start=True, stop=True)
                            ex = sbp.tile([128, 1024], bf16, tag="exps", bufs=3)
                            nc.scalar.activation(ex[:], ps[:], EXP, scale=SCALE)
                            for u in range(2):
                                jb = 2 * jbp + u
                                nc.tensor.matmul(
                                    po[0:65, :],
                                    vq[:, jb * 260 + hq * 65:
                                       jb * 260 + hq * 65 + 65],
                                    ex[:, u * 512:(u + 1) * 512],
                                    start=(jb == 0), stop=(jb == NJB - 1))
                            if jbp in (2, 5) and pending_norm:
                                pending_norm.pop(0)()
                            elif (p + 1) in gens:
                                pump(p + 1)
                            else:
                                pump("C")
                        # defer the normalization so the reciprocal overlaps
                        # the next head's S/av matmuls instead of stalling PE
                        pending_norm.append(_make_norm(po, outTh, head, iq))
                drain(p + 1)

            for fn in pending_norm:
                fn()
            pending_norm = []
            drain("C")

            # --- phase C pass 2: chunks 6..7 + bias ----------------------
            outc67 = []
            for cc in (6, 7):
                oc = sbp.tile([128, NI], bf16, tag="xT", bufs=8,
                              name=f"outc{cc}")
                nc.gpsimd.dma_start(out=oc[0:64, :], in_=sc_d[2 * cc])
                nc.gpsimd.dma_start(out=oc[64:128, :], in_=sc_d[2 * cc + 1])
                outc67.append(oc)
            for ib2 in range(NI // 128):
                for eh in range(C // 512):
                    py = psp.tile([128, 512], f32, tag="pst", bufs=2,
                                  name=f"py{ib2}_{eh}")
                    for ci, cc in enumerate((6, 7)):
                        nc.tensor.matmul(
                            py[:],
                            outc67[ci][:, ib2 * 128:(ib2 + 1) * 128],
                            wo_holder["hi"][:, (cc - 4) * 1024 + eh * 512:
                                            (cc - 4) * 1024 + eh * 512 + 512],
                            start=(ci == 0), stop=(ci == 1))
                    ys = sbp.tile([128, 512], f32, tag="zb", bufs=4,
                                  name=f"ys{ib2}_{eh}")
                    nc.vector.tensor_tensor(
                        out=ys[:], in0=py[:],
                        in1=y_acc[(ib2, eh)][:], op=ADD)
                    nc.sync.dma_start(
                        out=y_d[ib2 * 128:(ib2 + 1) * 128,
                                eh * 512:(eh + 1) * 512],
                        in_=ys[:])

    nc.compile()
    return nc


def _get_program():
    if "nc" not in _CACHE:
        _CACHE["nc"] = _build_program()
    return _CACHE["nc"]


def _make_in_maps(x, Wq, Wk, Wv, Wo, bo):
    import ml_dtypes
    x = np.ascontiguousarray(np.asarray(x, dtype=np.float32))
    Wq = np.ascontiguousarray(np.asarray(Wq, dtype=np.float32))
    Wk = np.ascontiguousarray(np.asarray(Wk, dtype=np.float32))
    Wv = np.ascontiguousarray(np.asarray(Wv, dtype=np.float32))
    Wo = np.ascontiguousarray(np.asarray(Wo, dtype=np.float32))
    bo = np.ascontiguousarray(np.asarray(bo, dtype=np.float32))
    in_maps = []
    for c in range(8):
        b, s = c // 2, c % 2
        # rows 0..1023 of the per-core x are that core's query rows
        xb = x[b]
        xc = np.concatenate(
            [xb[s * NI:(s + 1) * NI], xb[(1 - s) * NI:(2 - s) * NI]], axis=0)
        in_maps.append({"x": np.ascontiguousarray(xc.T.astype(ml_dtypes.bfloat16)),
                        "Wq": Wq, "Wk": Wk, "Wv": Wv, "Wo": Wo, "bo": bo})
    return in_maps


def _assemble(results):
    out = np.empty((B, NSEQ, C), dtype=np.float32)
    for c in range(8):
        b, s = c // 2, c % 2
        out[b, s * NI:(s + 1) * NI] = results[c]["y"]
    return out


def kernel(x, Wq, Wk, Wv, Wo, bo):
    from concourse.bass_utils import run_bass_kernel_spmd

    nc = _get_program()
    in_maps = _make_in_maps(x, Wq, Wk, Wv, Wo, bo)
    res = run_bass_kernel_spmd(nc, in_maps, list(range(8)))
    return _assemble(res.results)


def kernel_traced(x, Wq, Wk, Wv, Wo, bo):
    """Like kernel() but also neuron-profiles; returns (out, exec_time_ns)."""
    from concourse.bass_utils import run_bass_kernel_spmd

    nc = _get_program()
    in_maps = _make_in_maps(x, Wq, Wk, Wv, Wo, bo)
    res = run_bass_kernel_spmd(nc, in_maps, list(range(8)), trace=True)
    return _assemble(res.results), res.exec_time_ns
